# revision 24
# baseline (speedup 1.0000x reference)
"""Trainium2 Bass kernel for the CrossAttention problem (self-contained).

Strategy: shard the N=4096 query rows across 8 cores (512 rows/core for each
batch element; batch 0 = unconditional, batch 1 = conditional).  The tiny
77-token context projections K = ctx@Wk and V = ctx@Wv are precomputed on the
host (0.6 GFLOP) and replicated; the device kernel does the N-scale work:

  qT    = (scale*Wq)^T @ xT        [512, 1024]  (PE, cols 0:512 uc / 512: cond)
  simT  = k_gh^T q_h               [77, 512] per (group, head)   (PE)
  E     = exp(simT)                (Act; logits are small, no max-subtract)
  Zb    = ones77^T @ E             [77, 512] broadcast denominator (PE)
  attn  = E / Zb                   (DVE/Pool tensor_tensor divide)
  outT += v_gh^T @ attn            (PE, PSUM-chained over the 4 cond branches)
  yT    = Wo^T @ outT + bo         [320, 1024] -> f16 DMA out

The soft mask on cond branch 0 needs wm = w_dot*(t/50*4.6)*max(sim_c0) over
the FULL batch, so an AllGather(max) collective runs across the 8 cores.  It
is issued ~6us into the kernel (right after the branch-0 sims) on a separate
DMA queue, and branch 0 is finished last using the factorization
  exp(sim0 + wm*ae) = exp(sim0) * exp(wm*ae)
so the phase-0 sims are exp'ed early and never recomputed.
"""

import sys

sys.path.insert(0, "/opt/trn_rl_repo")

import numpy as np

import concourse.bass as bass
import concourse.tile as tile
from concourse import bacc, bass_utils, mybir

# problem constants (hardcoded per the harness contract)
H, DH, L, C = 8, 64, 77, 4
N, DQ, DC, INNER = 4096, 320, 768, 512
N_CORES = 8
NS = N // N_CORES          # query rows per core per batch element
NSB = 2 * NS               # both batch elements
SCALE = DH ** -0.5
W_DOT, TOTAL_STEP, SCHED = 1.0, 50, 4.6
NG = 5                     # context groups: 0=uc, 1=c0, 2=c1, 3=c2, 4=c3

F32 = mybir.dt.float32
F16 = mybir.dt.float16

LAST_RESULTS = None  # BassKernelResults of the most recent run (for test.py)
TRACE = False


def build_kernel(wdotw: float):
    nc = bacc.Bacc("TRN2", target_bir_lowering=False, debug=False, num_devices=N_CORES)

    # ---- DRAM I/O (host prepares exactly these layouts; identity-AP DMAs) ----
    d_xt = nc.dram_tensor("xt", [128, 3, NSB], F16, kind="ExternalInput")
    d_wq = nc.dram_tensor("wq", [128, 3, INNER], F16, kind="ExternalInput")
    d_kt = nc.dram_tensor("kt", [128, 4, NG, L], F16, kind="ExternalInput")
    d_v = nc.dram_tensor("v", [L, NG, INNER], F16, kind="ExternalInput")
    d_wo = nc.dram_tensor("wo", [128, 4, DQ], F16, kind="ExternalInput")
    d_bo = nc.dram_tensor("bo", [128, 3], F32, kind="ExternalInput")
    d_ae = nc.dram_tensor("ae", [L, H, NS], F16, kind="ExternalInput")
    d_yt = nc.dram_tensor("yt", [DQ, NSB], F16, kind="ExternalOutput")

    with tile.TileContext(nc) as tc:
        _emit(nc, tc, wdotw, d_xt, d_wq, d_kt, d_v, d_wo, d_bo, d_ae, d_yt)
    nc.compile()
    return nc


def _emit(nc, tc, wdotw, d_xt, d_wq, d_kt, d_v, d_wo, d_bo, d_ae, d_yt):
    from contextlib import ExitStack

    ctx = ExitStack()
    singles = ctx.enter_context(tc.tile_pool(name="singles", bufs=1))
    dram = ctx.enter_context(tc.tile_pool(name="dram", bufs=1, space="DRAM"))
    epool = ctx.enter_context(tc.tile_pool(name="epool", bufs=4))
    atpool = ctx.enter_context(tc.tile_pool(name="atpool", bufs=8))
    wkpool = ctx.enter_context(tc.tile_pool(name="wkpool", bufs=4))

    # ---- persistent SBUF tiles ----
    s_xt = singles.tile([128, 3, NSB], F16)
    s_wq = singles.tile([128, 3, INNER], F16)
    s_kt = singles.tile([128, 4, NG, L], F16)
    s_v = singles.tile([L, NG, INNER], F16)
    s_wo = singles.tile([128, 4, DQ], F16)
    s_bo = singles.tile([128, 3], F32)
    s_ae = singles.tile([L, H, NS], F16)
    s_qt = singles.tile([128, 4, NSB], F16)
    s_om = singles.tile([128, 4, NSB], F16)
    s_y = singles.tile([128, 3, NSB], F16)
    s_e0 = singles.tile([L, H, NS], F16)       # exp(sim_c0), pre-mask
    s_rzd = singles.tile([1, NG * H, NS], F32)  # 1/Z rows packed on partition 0
    s_lmax = singles.tile([L, H], F32)
    s_lm = singles.tile([L, 1], F32)
    s_maxrow8 = singles.tile([1, N_CORES * L], F32)
    s_wm = singles.tile([1, 1], F16)
    s_wmcol = singles.tile([L, 1], F32)
    s_zsel = singles.tile([L, H, H], F16)      # one-hot Z row selectors
    ones_row = singles.tile([1, L], F16)

    # ---- input DMA: critical path on sync, the rest on the scalar DGE ----
    nc.sync.dma_start(out=s_xt[:], in_=d_xt.ap())
    nc.sync.dma_start(out=s_wq[:], in_=d_wq.ap())
    nc.sync.dma_start(out=s_kt[:], in_=d_kt.ap())
    nc.scalar.dma_start(out=s_v[:], in_=d_v.ap())
    nc.scalar.dma_start(out=s_wo[:], in_=d_wo.ap())
    nc.scalar.dma_start(out=s_bo[:], in_=d_bo.ap())
    nc.scalar.dma_start(out=s_ae[:], in_=d_ae.ap())

    nc.vector.memset(s_zsel[:], 0.0)
    for j in range(H):
        nc.vector.memset(s_zsel[:, j, j:j + 1], 1.0)
    nc.vector.memset(ones_row[:], 1.0)

    # ---- PSUM pools: psim 2 banks + pmix 2 banks + p_pvc 4 banks = 8 ----
    psim = ctx.enter_context(tc.tile_pool(name="psim", bufs=1, space="PSUM"))
    pmix = ctx.enter_context(tc.tile_pool(name="pmix", bufs=1, space="PSUM"))
    psing = ctx.enter_context(tc.tile_pool(name="psing", bufs=1, space="PSUM"))
    p_pvc = psing.tile([128, 4, NS], F32)      # pv accumulators, 2 heads/bank
    pzp = ctx.enter_context(tc.tile_pool(name="pzp", bufs=1, space="PSUM"))

    def ksl(g, h):
        return s_kt[(h % 2) * 64:(h % 2) * 64 + 64, h // 2, g, :]

    def qsl(half, h):
        return s_qt[(h % 2) * 64:(h % 2) * 64 + 64, h // 2,
                    half * NS:(half + 1) * NS]

    def vsl(g, h):
        return s_v[:, g, h * 64:(h + 1) * 64]

    def qproj(half):
        for dc in range(4):
            p = pmix.tile([128, NS], F32, tag="mix")
            for kc in range(3):
                nc.tensor.matmul(
                    p[:],
                    s_wq[:, kc, dc * 128:(dc + 1) * 128],
                    s_xt[:, kc, half * NS:(half + 1) * NS],
                    start=(kc == 0), stop=(kc == 2),
                )
            nc.scalar.copy(s_qt[:, dc, half * NS:(half + 1) * NS], p[:])

    # ---- phase 0: cond q proj, branch-0 sims -> exp + max -> collective ----
    qproj(1)
    for hp in range(4):
        p = psim.tile([L, 2, NS], F32, tag="sim")
        nc.tensor.matmul(p[:, 0, :], ksl(1, 2 * hp), qsl(1, 2 * hp),
                         start=True, stop=True)
        nc.tensor.matmul(p[:, 1, :], ksl(1, 2 * hp + 1), qsl(1, 2 * hp + 1),
                         start=True, stop=True)
        nc.vector.reduce_max(out=s_lmax[:, 2 * hp:2 * hp + 2], in_=p[:],
                             axis=mybir.AxisListType.X)
        nc.scalar.activation(s_e0[:, 2 * hp:2 * hp + 2, :], p[:],
                             mybir.ActivationFunctionType.Exp)
    nc.vector.reduce_max(out=s_lm[:], in_=s_lmax[:], axis=mybir.AxisListType.X)
    nc.vector.tensor_scalar_mul(s_lm[:], s_lm[:], float(wdotw))

    cin = dram.tile([1, L], F32)
    cout = dram.tile([N_CORES, L], F32)
    nc.gpsimd.dma_start(out=cin.rearrange("one f -> f one"), in_=s_lm[:])
    nc.gpsimd.collective_compute(
        "AllGather", mybir.AluOpType.bypass,
        replica_groups=[list(range(N_CORES))],
        ins=[cin.opt()], outs=[cout.opt()],
    )

    # ---- uc q proj, then the four groups that don't need the mask ----
    qproj(0)

    rzbpool = ctx.enter_context(tc.tile_pool(name="rzbpool", bufs=4))
    rzfpool = ctx.enter_context(tc.tile_pool(name="rzfpool", bufs=2))

    def norm_pv(base, es, g, chain, pz):
        """Given the 4 exp pair-tiles of a group: recip, gather, bcast, mul, pv."""
        rzf = rzfpool.tile([H, NS], F32, tag="rzf")
        nc.vector.reciprocal_approx_fast(rzf[:], pz[:])
        nc.scalar.dma_start(out=s_rzd[0:1, base:base + H, :], in_=rzf[:])
        for h in range(H):
            rzb = rzbpool.tile([L, NS], F32, tag="rzb")
            nc.gpsimd.partition_broadcast(rzb[:], s_rzd[0:1, base + h, :],
                                          channels=L)
            a = atpool.tile([L, NS], F16, tag="at")
            eng = nc.vector if (h % 3 == 0) else nc.gpsimd
            eng.tensor_mul(a[:], es[h // 2][:, h % 2, :], rzb[:])
            sl = p_pvc[(h % 2) * 64:(h % 2) * 64 + 64, h // 2, :]
            if chain == "uc":
                nc.tensor.matmul(sl, vsl(g, h), a[:], start=True, stop=True)
            elif chain == "last":
                nc.tensor.matmul(sl, vsl(g, h), a[:], start=False, stop=True)
            else:
                nc.tensor.matmul(sl, vsl(g, h), a[:],
                                 start=(chain == "first"), stop=False)
        if chain == "uc":
            for hp in range(4):
                nc.vector.tensor_copy(s_om[:, hp, 0:NS], p_pvc[:, hp, :])

    def group(g, half, base, chain):
        """qk -> exp -> compact Z for all 8 heads, then normalize + pv."""
        es = []
        pz = pzp.tile([H, NS], F32, tag="pz")
        for hp in range(4):
            p = psim.tile([L, 2, NS], F32, tag="sim")
            nc.tensor.matmul(p[:, 0, :], ksl(g, 2 * hp), qsl(half, 2 * hp),
                             start=True, stop=True)
            nc.tensor.matmul(p[:, 1, :], ksl(g, 2 * hp + 1), qsl(half, 2 * hp + 1),
                             start=True, stop=True)
            e = epool.tile([L, 2, NS], F16, tag="e")
            nc.scalar.activation(e[:], p[:], mybir.ActivationFunctionType.Exp)
            es.append(e)
            for k in range(2):
                h = 2 * hp + k
                nc.tensor.matmul(pz[:], s_zsel[:, h, :], e[:, k, :],
                                 start=(h == 0), stop=(h == 7))
        norm_pv(base, es, g, chain, pz)

    group(0, 0, 0, "uc")

    def wo_half(half, pool):
        for oc in range(3):
            ow = 128 if oc < 2 else 64
            p = pool.tile([128, NS], F32, tag="mix")
            for kc in range(4):
                nc.tensor.matmul(
                    p[0:ow, :],
                    s_wo[:, kc, oc * 128:oc * 128 + ow],
                    s_om[:, kc, half * NS:(half + 1) * NS],
                    start=(kc == 0), stop=(kc == 3),
                )
            nc.scalar.add(s_y[0:ow, oc, half * NS:(half + 1) * NS], p[0:ow, :],
                          s_bo[0:ow, oc:oc + 1])
            nc.scalar.dma_start(
                out=d_yt.ap()[oc * 128:oc * 128 + ow, half * NS:(half + 1) * NS],
                in_=s_y[0:ow, oc, half * NS:(half + 1) * NS])

    wo_half(0, pmix)               # uc output, independent of branch 0
    group(2, 1, 8, "first")
    group(3, 1, 16, "mid")
    group(4, 1, 24, "mid")

    # ---- phase 4: wm from the collective, finish branch 0, cond output ----
    nc.gpsimd.dma_start(out=s_maxrow8[:], in_=cout.rearrange("r f -> (r f)"))
    nc.vector.reduce_max(out=s_wm[:], in_=s_maxrow8[:], axis=mybir.AxisListType.X)
    p_wm = pmix.tile([L, 1], F32, tag="mix")
    nc.tensor.matmul(p_wm[:], ones_row[:], s_wm[:], start=True, stop=True)
    nc.vector.tensor_copy(s_wmcol[:], p_wm[:])

    e0ps = []
    pz4 = pzp.tile([H, NS], F32, tag="pz")
    for hp in range(4):
        eae = epool.tile([L, 2, NS], F16, tag="e")
        nc.scalar.activation(eae[:], s_ae[:, 2 * hp:2 * hp + 2, :],
                             mybir.ActivationFunctionType.Exp, scale=s_wmcol[:])
        e0p = wkpool.tile([L, 2, NS], F16, tag="e0p")
        nc.gpsimd.tensor_mul(e0p[:], s_e0[:, 2 * hp:2 * hp + 2, :], eae[:])
        e0ps.append(e0p)
        for k in range(2):
            h = 2 * hp + k
            nc.tensor.matmul(pz4[:], s_zsel[:, h, :], e0p[:, k, :],
                             start=(h == 0), stop=(h == 7))
    norm_pv(32, e0ps, 1, "last", pz4)
    for hp in range(4):
        nc.vector.tensor_copy(s_om[:, hp, NS:NSB], p_pvc[:, hp, :])

    wo_half(1, pmix)
    ctx.pop_all().close()


_CACHE = {}


def kernel(x, uc_context, ck, cv, attn_extra, Wq, Wk, Wv, Wo, bo, t):
    global LAST_RESULTS
    x = np.asarray(x, np.float32)
    uc_context = np.asarray(uc_context, np.float32)
    ck = np.asarray(ck, np.float32)
    cv = np.asarray(cv, np.float32)
    attn_extra = np.asarray(attn_extra, np.float32)
    Wq = np.asarray(Wq, np.float32)
    Wk = np.asarray(Wk, np.float32)
    Wv = np.asarray(Wv, np.float32)
    Wo = np.asarray(Wo, np.float32)
    bo = np.asarray(bo, np.float32)
    tv = float(np.asarray(t))
    wdotw = W_DOT * (tv / TOTAL_STEP) * SCHED

    if wdotw not in _CACHE:
        _CACHE[wdotw] = build_kernel(wdotw)
    nc = _CACHE[wdotw]

    # ---- host-side input prep (tiny context projections + layout) ----
    ctxK = np.concatenate([uc_context[0][None], ck[:, 0]], axis=0)  # [5, 77, 768]
    ctxV = np.concatenate([uc_context[0][None], cv[:, 0]], axis=0)
    K = ctxK @ Wk                                   # [5, 77, 512]
    V = ctxV @ Wv
    V[1:] *= 1.0 / C

    # kt[d2, hp, g, l] = K[g, l, hp*128 + d2]
    kt = np.ascontiguousarray(K.transpose(2, 0, 1).reshape(4, 128, NG, L)
                              .transpose(1, 0, 2, 3)).astype(np.float16)
    # v[l, g, i] = V[g, l, i]
    v16 = np.ascontiguousarray(V.transpose(1, 0, 2)).astype(np.float16)

    wq_pad = np.zeros((3, 128, INNER), np.float32)
    wq_pad.reshape(384, INNER)[:DQ] = Wq * SCALE
    wq_pad = np.ascontiguousarray(wq_pad.transpose(1, 0, 2)).astype(np.float16)

    wo_pad = np.zeros((4, 128, DQ), np.float32)
    wo_pad.reshape(512, DQ)[:] = Wo
    wo_pad = np.ascontiguousarray(wo_pad.transpose(1, 0, 2)).astype(np.float16)

    bo_pad = np.zeros((3, 128), np.float32)
    bo_pad.reshape(384)[:DQ] = bo
    bo_pad = np.ascontiguousarray(bo_pad.T)

    in_maps = []
    for c in range(N_CORES):
        rows = slice(c * NS, (c + 1) * NS)
        xt = np.zeros((3, 128, NSB), np.float32)
        xt.reshape(384, NSB)[:DQ, :NS] = x[0, rows].T
        xt.reshape(384, NSB)[:DQ, NS:] = x[1, rows].T
        xt = np.ascontiguousarray(xt.transpose(1, 0, 2)).astype(np.float16)
        # ae[l, h, n] = attn_extra[h, rows[n], l]
        ae = np.ascontiguousarray(
            attn_extra[:, rows, :].transpose(2, 0, 1)).astype(np.float16)
        in_maps.append({
            "xt": xt, "wq": wq_pad, "kt": kt, "v": v16, "wo": wo_pad,
            "bo": bo_pad, "ae": ae,
        })

    import os as _os
    _tc = None
    if _os.environ.get("KERNEL_TRACE_ALL") == "1":
        _tc = list(range(N_CORES))
    res = bass_utils.run_bass_kernel_spmd(
        nc, in_maps, core_ids=list(range(N_CORES)), trace=TRACE, trace_cores=_tc,
    )
    LAST_RESULTS = res

    out = np.empty((2, N, DQ), np.float32)
    for c in range(N_CORES):
        rows = slice(c * NS, (c + 1) * NS)
        yt = res.results[c]["yt"].astype(np.float32)
        out[0, rows] = yt[:, :NS].T
        out[1, rows] = yt[:, NS:].T
    return out


# revision 40
# speedup vs baseline: 1.4816x; 1.4816x over previous
"""Trainium2 Bass kernel for the CrossAttention problem (self-contained).

Strategy: shard the N=4096 query rows across 8 cores (512 rows/core for each
batch element; batch 0 = unconditional, batch 1 = conditional).  The tiny
77-token context projections K = ctx@Wk and V = ctx@Wv are precomputed on the
host (0.6 GFLOP) and replicated; the device kernel does the N-scale work:

  qT    = (scale*Wq)^T @ xT        [512, 1024]  (PE, cols 0:512 uc / 512: cond)
  simT  = k_gh^T q_h               [77, 512] per (group, head)   (PE)
  E     = exp(simT)                (Act; logits are small, no max-subtract)
  Zb    = ones77^T @ E             [77, 512] broadcast denominator (PE)
  attn  = E / Zb                   (DVE/Pool tensor_tensor divide)
  outT += v_gh^T @ attn            (PE, PSUM-chained over the 4 cond branches)
  yT    = Wo^T @ outT + bo         [320, 1024] -> f16 DMA out

The soft mask on cond branch 0 needs wm = w_dot*(t/50*4.6)*max(sim_c0) over
the FULL batch, so an AllGather(max) collective runs across the 8 cores.  It
is issued ~6us into the kernel (right after the branch-0 sims) on a separate
DMA queue, and branch 0 is finished last using the factorization
  exp(sim0 + wm*ae) = exp(sim0) * exp(wm*ae)
so the phase-0 sims are exp'ed early and never recomputed.
"""

import sys

sys.path.insert(0, "/opt/trn_rl_repo")

import numpy as np

import concourse.bass as bass
import concourse.tile as tile
from concourse import bacc, bass_utils, mybir

# problem constants (hardcoded per the harness contract)
H, DH, L, C = 8, 64, 77, 4
N, DQ, DC, INNER = 4096, 320, 768, 512
N_CORES = 8
NS = N // N_CORES          # query rows per core per batch element
NSB = 2 * NS               # both batch elements
SCALE = DH ** -0.5
W_DOT, TOTAL_STEP, SCHED = 1.0, 50, 4.6
NG = 5                     # context groups: 0=uc, 1=c0, 2=c1, 3=c2, 4=c3

F32 = mybir.dt.float32
F16 = mybir.dt.float16

LAST_RESULTS = None  # BassKernelResults of the most recent run (for test.py)
TRACE = False


def build_kernel(wdotw: float):
    nc = bacc.Bacc("TRN2", target_bir_lowering=False, debug=False, num_devices=N_CORES)

    # ---- DRAM I/O (host prepares exactly these layouts; identity-AP DMAs) ----
    d_xt = nc.dram_tensor("xt", [128, 3, NSB], F16, kind="ExternalInput")
    d_wq = nc.dram_tensor("wq", [128, 3, INNER], F16, kind="ExternalInput")
    d_kt = nc.dram_tensor("kt", [128, 4, NG, L], F16, kind="ExternalInput")
    d_v = nc.dram_tensor("v", [L, NG, INNER], F16, kind="ExternalInput")
    d_wo = nc.dram_tensor("wo", [128, 4, DQ], F16, kind="ExternalInput")
    d_bo = nc.dram_tensor("bo", [128, 3], F32, kind="ExternalInput")
    d_ae = nc.dram_tensor("ae", [L, H, NS], F16, kind="ExternalInput")
    d_yt = nc.dram_tensor("yt", [DQ, NSB], F16, kind="ExternalOutput")

    with tile.TileContext(nc) as tc:
        _emit(nc, tc, wdotw, d_xt, d_wq, d_kt, d_v, d_wo, d_bo, d_ae, d_yt)
    nc.compile()
    return nc


def _emit(nc, tc, wdotw, d_xt, d_wq, d_kt, d_v, d_wo, d_bo, d_ae, d_yt):
    from contextlib import ExitStack

    ctx = ExitStack()
    singles = ctx.enter_context(tc.tile_pool(name="singles", bufs=1))
    dram = ctx.enter_context(tc.tile_pool(name="dram", bufs=1, space="DRAM"))
    epool = ctx.enter_context(tc.tile_pool(name="epool", bufs=4))
    atpool = ctx.enter_context(tc.tile_pool(name="atpool", bufs=8))
    wkpool = ctx.enter_context(tc.tile_pool(name="wkpool", bufs=4))

    # ---- persistent SBUF tiles ----
    s_xt = singles.tile([128, 3, NSB], F16)
    s_wq = singles.tile([128, 3, INNER], F16)
    s_kt = singles.tile([128, 4, NG, L], F16)
    s_v = singles.tile([L, NG, INNER], F16)
    s_wo = singles.tile([128, 4, DQ], F16)
    s_bo = singles.tile([128, 3], F32)
    s_ae = singles.tile([L, H, NS], F16)
    s_qt = singles.tile([128, 4, NSB], F16)
    s_om = singles.tile([128, 4, NSB], F16)
    s_y = singles.tile([128, 3, NSB], F16)
    s_e0 = singles.tile([L, H, NS], F16)       # exp(sim_c0), pre-mask
    s_lmax = singles.tile([L, H], F32)
    s_lm = singles.tile([L, 1], F32)
    s_maxrow8 = singles.tile([1, N_CORES * L], F32)
    s_wm = singles.tile([1, 1], F16)
    s_wmcol = singles.tile([L, 1], F32)
    ones77 = singles.tile([L, L], F16)
    ones_row = singles.tile([1, L], F16)

    # ---- input DMA: critical path on sync, the rest on the scalar DGE ----
    nc.sync.dma_start(out=s_xt[:], in_=d_xt.ap())
    nc.sync.dma_start(out=s_wq[:], in_=d_wq.ap())
    nc.sync.dma_start(out=s_kt[:], in_=d_kt.ap())
    nc.scalar.dma_start(out=s_v[:], in_=d_v.ap())
    nc.scalar.dma_start(out=s_wo[:], in_=d_wo.ap())
    nc.scalar.dma_start(out=s_bo[:], in_=d_bo.ap())
    nc.scalar.dma_start(out=s_ae[:], in_=d_ae.ap())

    nc.vector.memset(ones77[:], 1.0)
    nc.vector.memset(ones_row[:], 1.0)

    # ---- PSUM pools: psim 2 banks + pmix 2 banks + p_pvc 4 banks = 8 ----
    psim = ctx.enter_context(tc.tile_pool(name="psim", bufs=1, space="PSUM"))
    psing = ctx.enter_context(tc.tile_pool(name="psing", bufs=1, space="PSUM"))
    p_pvc = psing.tile([128, 4, NS], F32)      # pv accumulators, 2 heads/bank
    pzb = ctx.enter_context(tc.tile_pool(name="pzb", bufs=1, space="PSUM"))

    def ksl(g, h):
        return s_kt[(h % 2) * 64:(h % 2) * 64 + 64, h // 2, g, :]

    def qsl(half, h):
        return s_qt[(h % 2) * 64:(h % 2) * 64 + 64, h // 2,
                    half * NS:(half + 1) * NS]

    def vsl(g, h):
        return s_v[:, g, h * 64:(h + 1) * 64]

    def qproj(half):
        for dc in range(4):
            p = psim.tile([128, NS], F32, tag="sim")
            for kc in range(3):
                nc.tensor.matmul(
                    p[:],
                    s_wq[:, kc, dc * 128:(dc + 1) * 128],
                    s_xt[:, kc, half * NS:(half + 1) * NS],
                    start=(kc == 0), stop=(kc == 2),
                )
            nc.scalar.copy(s_qt[:, dc, half * NS:(half + 1) * NS], p[:])

    # ---- phase 0: cond q proj, branch-0 sims -> exp + max -> collective ----
    qproj(1)
    for hp in range(4):
        p = psim.tile([L, 2, NS], F32, tag="sim")
        nc.tensor.matmul(p[:, 0, :], ksl(1, 2 * hp), qsl(1, 2 * hp),
                         start=True, stop=True)
        nc.tensor.matmul(p[:, 1, :], ksl(1, 2 * hp + 1), qsl(1, 2 * hp + 1),
                         start=True, stop=True)
        nc.vector.reduce_max(out=s_lmax[:, 2 * hp:2 * hp + 2], in_=p[:],
                             axis=mybir.AxisListType.X)
        nc.scalar.activation(s_e0[:, 2 * hp:2 * hp + 2, :], p[:],
                             mybir.ActivationFunctionType.Exp)
    nc.vector.reduce_max(out=s_lm[:], in_=s_lmax[:], axis=mybir.AxisListType.X)
    nc.vector.tensor_scalar_mul(s_lm[:], s_lm[:], float(wdotw))

    cin = dram.tile([1, L], F32)
    cout = dram.tile([N_CORES, L], F32)
    nc.gpsimd.dma_start(out=cin.rearrange("one f -> f one"), in_=s_lm[:])
    nc.gpsimd.collective_compute(
        "AllGather", mybir.AluOpType.bypass,
        replica_groups=[list(range(N_CORES))],
        ins=[cin.opt()], outs=[cout.opt()],
    )

    # ---- uc q proj, then the four groups that don't need the mask ----
    qproj(0)

    def pv_mm(g, h, a, chain):
        sl = p_pvc[(h % 2) * 64:(h % 2) * 64 + 64, h // 2, :]
        if chain == "uc":
            nc.tensor.matmul(sl, vsl(g, h), a[:], start=True, stop=True)
        elif chain == "last":
            nc.tensor.matmul(sl, vsl(g, h), a[:], start=False, stop=True)
        else:
            nc.tensor.matmul(sl, vsl(g, h), a[:],
                             start=(chain == "first"), stop=False)

    def group(g, half, chain):
        """qk -> exp -> broadcast Z -> divide -> pv for the 8 heads."""
        for hp in range(4):
            p = psim.tile([L, 2, NS], F32, tag="sim")
            nc.tensor.matmul(p[:, 0, :], ksl(g, 2 * hp), qsl(half, 2 * hp),
                             start=True, stop=True)
            nc.tensor.matmul(p[:, 1, :], ksl(g, 2 * hp + 1), qsl(half, 2 * hp + 1),
                             start=True, stop=True)
            e = epool.tile([L, 2, NS], F16, tag="e")
            nc.scalar.activation(e[:], p[:], mybir.ActivationFunctionType.Exp)
            z = pzb.tile([L, 2, NS], F32, tag="zb")
            nc.tensor.matmul(z[:, 0, :], ones77[:], e[:, 0, :],
                             start=True, stop=True)
            nc.tensor.matmul(z[:, 1, :], ones77[:], e[:, 1, :],
                             start=True, stop=True)
            rz = wkpool.tile([L, 2, NS], F32, tag="rz")
            nc.vector.reciprocal_approx_fast(rz[:], z[:])
            for k in range(2):
                h = 2 * hp + k
                a = atpool.tile([L, NS], F16, tag="at")
                eng = nc.vector if (h % 4 == 0) else nc.gpsimd
                eng.tensor_mul(a[:], e[:, k, :], rz[:, k, :])
                pv_mm(g, h, a, chain)
        if chain == "uc":
            for hp in range(4):
                nc.vector.tensor_copy(s_om[:, hp, 0:NS], p_pvc[:, hp, :])

    group(0, 0, "uc")

    def wo_half(half, pool):
        for oc in range(3):
            ow = 128 if oc < 2 else 64
            p = pool.tile([128, NS], F32, tag="sim")
            for kc in range(4):
                nc.tensor.matmul(
                    p[0:ow, :],
                    s_wo[:, kc, oc * 128:oc * 128 + ow],
                    s_om[:, kc, half * NS:(half + 1) * NS],
                    start=(kc == 0), stop=(kc == 3),
                )
            nc.scalar.add(s_y[0:ow, oc, half * NS:(half + 1) * NS], p[0:ow, :],
                          s_bo[0:ow, oc:oc + 1])
            nc.scalar.dma_start(
                out=d_yt.ap()[oc * 128:oc * 128 + ow, half * NS:(half + 1) * NS],
                in_=s_y[0:ow, oc, half * NS:(half + 1) * NS])

    wo_half(0, psim)               # uc output, independent of branch 0
    group(2, 1, "first")
    group(3, 1, "mid")
    group(4, 1, "mid")

    # ---- phase 4: wm from the collective, finish branch 0, cond output ----
    nc.gpsimd.dma_start(out=s_maxrow8[:], in_=cout.rearrange("r f -> (r f)"))
    nc.vector.reduce_max(out=s_wm[:], in_=s_maxrow8[:], axis=mybir.AxisListType.X)
    p_wm = psim.tile([L, 1], F32, tag="sim")
    nc.tensor.matmul(p_wm[:], ones_row[:], s_wm[:], start=True, stop=True)
    nc.vector.tensor_copy(s_wmcol[:], p_wm[:])

    for hp in range(4):
        eae = epool.tile([L, 2, NS], F16, tag="e")
        nc.scalar.activation(eae[:], s_ae[:, 2 * hp:2 * hp + 2, :],
                             mybir.ActivationFunctionType.Exp, scale=s_wmcol[:])
        e0p = wkpool.tile([L, 2, NS], F16, tag="e0p")
        nc.gpsimd.tensor_mul(e0p[:], s_e0[:, 2 * hp:2 * hp + 2, :], eae[:])
        z = pzb.tile([L, 2, NS], F32, tag="zb")
        nc.tensor.matmul(z[:, 0, :], ones77[:], e0p[:, 0, :], start=True, stop=True)
        nc.tensor.matmul(z[:, 1, :], ones77[:], e0p[:, 1, :], start=True, stop=True)
        rz = wkpool.tile([L, 2, NS], F32, tag="rz")
        nc.vector.reciprocal_approx_fast(rz[:], z[:])
        for k in range(2):
            h = 2 * hp + k
            a = atpool.tile([L, NS], F16, tag="at")
            eng = nc.vector if (h % 4 == 0) else nc.gpsimd
            eng.tensor_mul(a[:], e0p[:, k, :], rz[:, k, :])
            pv_mm(1, h, a, "last")
        nc.vector.tensor_copy(s_om[:, hp, NS:NSB], p_pvc[:, hp, :])

    wo_half(1, psim)
    ctx.pop_all().close()


_CACHE = {}


def kernel(x, uc_context, ck, cv, attn_extra, Wq, Wk, Wv, Wo, bo, t):
    global LAST_RESULTS
    x = np.asarray(x, np.float32)
    uc_context = np.asarray(uc_context, np.float32)
    ck = np.asarray(ck, np.float32)
    cv = np.asarray(cv, np.float32)
    attn_extra = np.asarray(attn_extra, np.float32)
    Wq = np.asarray(Wq, np.float32)
    Wk = np.asarray(Wk, np.float32)
    Wv = np.asarray(Wv, np.float32)
    Wo = np.asarray(Wo, np.float32)
    bo = np.asarray(bo, np.float32)
    tv = float(np.asarray(t))
    wdotw = W_DOT * (tv / TOTAL_STEP) * SCHED

    if wdotw not in _CACHE:
        _CACHE[wdotw] = build_kernel(wdotw)
    nc = _CACHE[wdotw]

    # ---- host-side input prep (tiny context projections + layout) ----
    ctxK = np.concatenate([uc_context[0][None], ck[:, 0]], axis=0)  # [5, 77, 768]
    ctxV = np.concatenate([uc_context[0][None], cv[:, 0]], axis=0)
    K = ctxK @ Wk                                   # [5, 77, 512]
    V = ctxV @ Wv
    V[1:] *= 1.0 / C

    # kt[d2, hp, g, l] = K[g, l, hp*128 + d2]
    kt = np.ascontiguousarray(K.transpose(2, 0, 1).reshape(4, 128, NG, L)
                              .transpose(1, 0, 2, 3)).astype(np.float16)
    # v[l, g, i] = V[g, l, i]
    v16 = np.ascontiguousarray(V.transpose(1, 0, 2)).astype(np.float16)

    wq_pad = np.zeros((3, 128, INNER), np.float32)
    wq_pad.reshape(384, INNER)[:DQ] = Wq * SCALE
    wq_pad = np.ascontiguousarray(wq_pad.transpose(1, 0, 2)).astype(np.float16)

    wo_pad = np.zeros((4, 128, DQ), np.float32)
    wo_pad.reshape(512, DQ)[:] = Wo
    wo_pad = np.ascontiguousarray(wo_pad.transpose(1, 0, 2)).astype(np.float16)

    bo_pad = np.zeros((3, 128), np.float32)
    bo_pad.reshape(384)[:DQ] = bo
    bo_pad = np.ascontiguousarray(bo_pad.T)

    in_maps = []
    for c in range(N_CORES):
        rows = slice(c * NS, (c + 1) * NS)
        xt = np.zeros((3, 128, NSB), np.float32)
        xt.reshape(384, NSB)[:DQ, :NS] = x[0, rows].T
        xt.reshape(384, NSB)[:DQ, NS:] = x[1, rows].T
        xt = np.ascontiguousarray(xt.transpose(1, 0, 2)).astype(np.float16)
        # ae[l, h, n] = attn_extra[h, rows[n], l]
        ae = np.ascontiguousarray(
            attn_extra[:, rows, :].transpose(2, 0, 1)).astype(np.float16)
        in_maps.append({
            "xt": xt, "wq": wq_pad, "kt": kt, "v": v16, "wo": wo_pad,
            "bo": bo_pad, "ae": ae,
        })

    import os as _os
    _tc = None
    if _os.environ.get("KERNEL_TRACE_ALL") == "1":
        _tc = list(range(N_CORES))
    res = bass_utils.run_bass_kernel_spmd(
        nc, in_maps, core_ids=list(range(N_CORES)), trace=TRACE, trace_cores=_tc,
    )
    LAST_RESULTS = res

    out = np.empty((2, N, DQ), np.float32)
    for c in range(N_CORES):
        rows = slice(c * NS, (c + 1) * NS)
        yt = res.results[c]["yt"].astype(np.float32)
        out[0, rows] = yt[:, :NS].T
        out[1, rows] = yt[:, NS:].T
    return out


# revision 41
# speedup vs baseline: 2.3126x; 1.5609x over previous
"""Trainium2 Bass kernel for the CrossAttention problem (self-contained).

Strategy: shard the N=4096 query rows across 8 cores (512 rows/core for each
batch element; batch 0 = unconditional, batch 1 = conditional).  The tiny
77-token context projections K = ctx@Wk and V = ctx@Wv are precomputed on the
host (0.6 GFLOP) and replicated; the device kernel does the N-scale work:

  qT    = (scale*Wq)^T @ xT        [512, 1024]  (PE, cols 0:512 uc / 512: cond)
  simT  = k_gh^T q_h               [77, 512] per (group, head)   (PE)
  E     = exp(simT)                (Act; logits are small, no max-subtract)
  Zb    = ones77^T @ E             [77, 512] broadcast denominator (PE)
  attn  = E / Zb                   (DVE/Pool tensor_tensor divide)
  outT += v_gh^T @ attn            (PE, PSUM-chained over the 4 cond branches)
  yT    = Wo^T @ outT + bo         [320, 1024] -> f16 DMA out

The soft mask on cond branch 0 needs wm = w_dot*(t/50*4.6)*max(sim_c0) over
the FULL batch, so an AllGather(max) collective runs across the 8 cores.  It
is issued ~6us into the kernel (right after the branch-0 sims) on a separate
DMA queue, and branch 0 is finished last using the factorization
  exp(sim0 + wm*ae) = exp(sim0) * exp(wm*ae)
so the phase-0 sims are exp'ed early and never recomputed.
"""

import sys

sys.path.insert(0, "/opt/trn_rl_repo")

import numpy as np

import concourse.bass as bass
import concourse.tile as tile
from concourse import bacc, bass_utils, mybir

# problem constants (hardcoded per the harness contract)
H, DH, L, C = 8, 64, 77, 4
N, DQ, DC, INNER = 4096, 320, 768, 512
N_CORES = 8
NS = N // N_CORES          # query rows per core per batch element
NSB = 2 * NS               # both batch elements
SCALE = DH ** -0.5
W_DOT, TOTAL_STEP, SCHED = 1.0, 50, 4.6
NG = 5                     # context groups: 0=uc, 1=c0, 2=c1, 3=c2, 4=c3

F32 = mybir.dt.float32
F16 = mybir.dt.float16

LAST_RESULTS = None  # BassKernelResults of the most recent run (for test.py)
TRACE = False


def build_kernel(wdotw: float):
    nc = bacc.Bacc("TRN2", target_bir_lowering=False, debug=False, num_devices=N_CORES)

    # ---- DRAM I/O (host prepares exactly these layouts; identity-AP DMAs) ----
    d_xt = nc.dram_tensor("xt", [128, 3, NSB], F16, kind="ExternalInput")
    d_wq = nc.dram_tensor("wq", [128, 3, INNER], F16, kind="ExternalInput")
    d_kt = nc.dram_tensor("kt", [128, 4, NG, L], F16, kind="ExternalInput")
    d_v = nc.dram_tensor("v", [L, NG, INNER], F16, kind="ExternalInput")
    d_wo = nc.dram_tensor("wo", [128, 4, DQ], F16, kind="ExternalInput")
    d_bo = nc.dram_tensor("bo", [128, 3], F32, kind="ExternalInput")
    d_ae = nc.dram_tensor("ae", [L, H, NS], F16, kind="ExternalInput")
    d_yt = nc.dram_tensor("yt", [DQ, NSB], F16, kind="ExternalOutput")

    with tile.TileContext(nc) as tc:
        _emit(nc, tc, wdotw, d_xt, d_wq, d_kt, d_v, d_wo, d_bo, d_ae, d_yt)
    nc.compile()
    return nc


def _emit(nc, tc, wdotw, d_xt, d_wq, d_kt, d_v, d_wo, d_bo, d_ae, d_yt):
    from contextlib import ExitStack

    ctx = ExitStack()
    singles = ctx.enter_context(tc.tile_pool(name="singles", bufs=1))
    dram = ctx.enter_context(tc.tile_pool(name="dram", bufs=1, space="DRAM"))
    epool = ctx.enter_context(tc.tile_pool(name="epool", bufs=4))
    atpool = ctx.enter_context(tc.tile_pool(name="atpool", bufs=8))
    wkpool = ctx.enter_context(tc.tile_pool(name="wkpool", bufs=4))

    # ---- persistent SBUF tiles ----
    s_xt = singles.tile([128, 3, NSB], F16)
    s_wq = singles.tile([128, 3, INNER], F16)
    s_kt = singles.tile([128, 4, NG, L], F16)
    s_v = singles.tile([L, NG, INNER], F16)
    s_wo = singles.tile([128, 4, DQ], F16)
    s_bo = singles.tile([128, 3], F32)
    s_ae = singles.tile([L, H, NS], F16)
    s_qt = singles.tile([128, 4, NSB], F16)
    s_om = singles.tile([128, 4, NSB], F16)
    s_y = singles.tile([128, 3, NSB], F16)
    s_e0 = singles.tile([L, H, NS], F16)       # exp(sim_c0), pre-mask
    s_lmax = singles.tile([L, H], F32)
    s_lm = singles.tile([L, 1], F32)
    s_maxrow8 = singles.tile([1, N_CORES * L], F32)
    s_wm = singles.tile([1, 1], F16)
    s_wmcol = singles.tile([L, 1], F32)
    ones77 = singles.tile([L, L], F16)
    ones_row = singles.tile([1, L], F16)

    # ---- input DMA: critical path on sync, the rest on the scalar DGE ----
    nc.sync.dma_start(out=s_xt[:], in_=d_xt.ap())
    nc.sync.dma_start(out=s_wq[:], in_=d_wq.ap())
    nc.sync.dma_start(out=s_kt[:], in_=d_kt.ap())
    nc.scalar.dma_start(out=s_v[:], in_=d_v.ap())
    nc.scalar.dma_start(out=s_wo[:], in_=d_wo.ap())
    nc.scalar.dma_start(out=s_bo[:], in_=d_bo.ap())
    nc.scalar.dma_start(out=s_ae[:], in_=d_ae.ap())

    nc.vector.memset(ones77[:], 1.0)
    nc.vector.memset(ones_row[:], 1.0)

    # ---- PSUM pools: psim 2 banks + pmix 2 banks + p_pvc 4 banks = 8 ----
    psim = ctx.enter_context(tc.tile_pool(name="psim", bufs=1, space="PSUM"))
    psing = ctx.enter_context(tc.tile_pool(name="psing", bufs=1, space="PSUM"))
    p_pvc = psing.tile([128, 4, NS], F32)      # pv accumulators, 2 heads/bank
    pzb = ctx.enter_context(tc.tile_pool(name="pzb", bufs=1, space="PSUM"))

    def ksl(g, h):
        return s_kt[(h % 2) * 64:(h % 2) * 64 + 64, h // 2, g, :]

    def qsl(half, h):
        return s_qt[(h % 2) * 64:(h % 2) * 64 + 64, h // 2,
                    half * NS:(half + 1) * NS]

    def vsl(g, h):
        return s_v[:, g, h * 64:(h + 1) * 64]

    def qproj(half):
        for dc in range(4):
            p = psim.tile([128, NS], F32, tag="sim")
            for kc in range(3):
                nc.tensor.matmul(
                    p[:],
                    s_wq[:, kc, dc * 128:(dc + 1) * 128],
                    s_xt[:, kc, half * NS:(half + 1) * NS],
                    start=(kc == 0), stop=(kc == 2),
                )
            nc.scalar.copy(s_qt[:, dc, half * NS:(half + 1) * NS], p[:])

    # ---- phase 0: cond q proj, branch-0 sims -> exp + max -> collective ----
    qproj(1)
    for hp in range(4):
        p = psim.tile([L, 2, NS], F32, tag="sim")
        nc.tensor.matmul(p[:, 0, :], ksl(1, 2 * hp), qsl(1, 2 * hp),
                         start=True, stop=True)
        nc.tensor.matmul(p[:, 1, :], ksl(1, 2 * hp + 1), qsl(1, 2 * hp + 1),
                         start=True, stop=True)
        nc.vector.reduce_max(out=s_lmax[:, 2 * hp:2 * hp + 2], in_=p[:],
                             axis=mybir.AxisListType.X)
        nc.scalar.activation(s_e0[:, 2 * hp:2 * hp + 2, :], p[:],
                             mybir.ActivationFunctionType.Exp)
    nc.vector.reduce_max(out=s_lm[:], in_=s_lmax[:], axis=mybir.AxisListType.X)
    nc.vector.tensor_scalar_mul(s_lm[:], s_lm[:], float(wdotw))

    cin = dram.tile([1, L], F32)
    cout = dram.tile([N_CORES, L], F32)
    nc.gpsimd.dma_start(out=cin.rearrange("one f -> f one"), in_=s_lm[:])
    nc.gpsimd.collective_compute(
        "AllGather", mybir.AluOpType.bypass,
        replica_groups=[list(range(N_CORES))],
        ins=[cin.opt()], outs=[cout.opt()],
    )

    # ---- uc q proj, then the four groups that don't need the mask ----
    qproj(0)

    def pv_mm(g, h, a, chain):
        sl = p_pvc[(h % 2) * 64:(h % 2) * 64 + 64, h // 2, :]
        if chain == "uc":
            nc.tensor.matmul(sl, vsl(g, h), a[:], start=True, stop=True)
        elif chain == "last":
            nc.tensor.matmul(sl, vsl(g, h), a[:], start=False, stop=True)
        else:
            nc.tensor.matmul(sl, vsl(g, h), a[:],
                             start=(chain == "first"), stop=False)

    def group(g, half, chain):
        """qk -> exp -> broadcast Z -> divide -> pv for the 8 heads."""
        for hp in range(4):
            p = psim.tile([L, 2, NS], F32, tag="sim")
            nc.tensor.matmul(p[:, 0, :], ksl(g, 2 * hp), qsl(half, 2 * hp),
                             start=True, stop=True)
            nc.tensor.matmul(p[:, 1, :], ksl(g, 2 * hp + 1), qsl(half, 2 * hp + 1),
                             start=True, stop=True)
            e = epool.tile([L, 2, NS], F16, tag="e")
            nc.scalar.activation(e[:], p[:], mybir.ActivationFunctionType.Exp)
            z = pzb.tile([L, 2, NS], F32, tag="zb")
            nc.tensor.matmul(z[:, 0, :], ones77[:], e[:, 0, :],
                             start=True, stop=True)
            nc.tensor.matmul(z[:, 1, :], ones77[:], e[:, 1, :],
                             start=True, stop=True)
            rz = wkpool.tile([L, 2, NS], F32, tag="rz")
            nc.vector.reciprocal_approx_fast(rz[:], z[:])
            for k in range(2):
                h = 2 * hp + k
                a = atpool.tile([L, NS], F16, tag="at")
                nc.vector.tensor_mul(a[:], e[:, k, :], rz[:, k, :])
                pv_mm(g, h, a, chain)
        if chain == "uc":
            for hp in range(4):
                nc.scalar.copy(s_om[:, hp, 0:NS], p_pvc[:, hp, :])

    group(0, 0, "uc")

    def wo_half(half, pool):
        for oc in range(3):
            ow = 128 if oc < 2 else 64
            p = pool.tile([128, NS], F32, tag="sim")
            for kc in range(4):
                nc.tensor.matmul(
                    p[0:ow, :],
                    s_wo[:, kc, oc * 128:oc * 128 + ow],
                    s_om[:, kc, half * NS:(half + 1) * NS],
                    start=(kc == 0), stop=(kc == 3),
                )
            nc.scalar.add(s_y[0:ow, oc, half * NS:(half + 1) * NS], p[0:ow, :],
                          s_bo[0:ow, oc:oc + 1])
            nc.scalar.dma_start(
                out=d_yt.ap()[oc * 128:oc * 128 + ow, half * NS:(half + 1) * NS],
                in_=s_y[0:ow, oc, half * NS:(half + 1) * NS])

    wo_half(0, psim)               # uc output, independent of branch 0
    group(2, 1, "first")
    group(3, 1, "mid")
    group(4, 1, "mid")

    # ---- phase 4: wm from the collective, finish branch 0, cond output ----
    nc.gpsimd.dma_start(out=s_maxrow8[:], in_=cout.rearrange("r f -> (r f)"))
    nc.vector.reduce_max(out=s_wm[:], in_=s_maxrow8[:], axis=mybir.AxisListType.X)
    p_wm = psim.tile([L, 1], F32, tag="sim")
    nc.tensor.matmul(p_wm[:], ones_row[:], s_wm[:], start=True, stop=True)
    nc.vector.tensor_copy(s_wmcol[:], p_wm[:])

    for hp in range(4):
        eae = epool.tile([L, 2, NS], F16, tag="e")
        nc.scalar.activation(eae[:], s_ae[:, 2 * hp:2 * hp + 2, :],
                             mybir.ActivationFunctionType.Exp, scale=s_wmcol[:])
        e0p = wkpool.tile([L, 2, NS], F16, tag="e0p")
        nc.vector.tensor_mul(e0p[:], s_e0[:, 2 * hp:2 * hp + 2, :], eae[:])
        z = pzb.tile([L, 2, NS], F32, tag="zb")
        nc.tensor.matmul(z[:, 0, :], ones77[:], e0p[:, 0, :], start=True, stop=True)
        nc.tensor.matmul(z[:, 1, :], ones77[:], e0p[:, 1, :], start=True, stop=True)
        rz = wkpool.tile([L, 2, NS], F32, tag="rz")
        nc.vector.reciprocal_approx_fast(rz[:], z[:])
        for k in range(2):
            h = 2 * hp + k
            a = atpool.tile([L, NS], F16, tag="at")
            nc.vector.tensor_mul(a[:], e0p[:, k, :], rz[:, k, :])
            pv_mm(1, h, a, "last")
        nc.scalar.copy(s_om[:, hp, NS:NSB], p_pvc[:, hp, :])

    wo_half(1, psim)
    ctx.pop_all().close()


_CACHE = {}


def kernel(x, uc_context, ck, cv, attn_extra, Wq, Wk, Wv, Wo, bo, t):
    global LAST_RESULTS
    x = np.asarray(x, np.float32)
    uc_context = np.asarray(uc_context, np.float32)
    ck = np.asarray(ck, np.float32)
    cv = np.asarray(cv, np.float32)
    attn_extra = np.asarray(attn_extra, np.float32)
    Wq = np.asarray(Wq, np.float32)
    Wk = np.asarray(Wk, np.float32)
    Wv = np.asarray(Wv, np.float32)
    Wo = np.asarray(Wo, np.float32)
    bo = np.asarray(bo, np.float32)
    tv = float(np.asarray(t))
    wdotw = W_DOT * (tv / TOTAL_STEP) * SCHED

    if wdotw not in _CACHE:
        _CACHE[wdotw] = build_kernel(wdotw)
    nc = _CACHE[wdotw]

    # ---- host-side input prep (tiny context projections + layout) ----
    ctxK = np.concatenate([uc_context[0][None], ck[:, 0]], axis=0)  # [5, 77, 768]
    ctxV = np.concatenate([uc_context[0][None], cv[:, 0]], axis=0)
    K = ctxK @ Wk                                   # [5, 77, 512]
    V = ctxV @ Wv
    V[1:] *= 1.0 / C

    # kt[d2, hp, g, l] = K[g, l, hp*128 + d2]
    kt = np.ascontiguousarray(K.transpose(2, 0, 1).reshape(4, 128, NG, L)
                              .transpose(1, 0, 2, 3)).astype(np.float16)
    # v[l, g, i] = V[g, l, i]
    v16 = np.ascontiguousarray(V.transpose(1, 0, 2)).astype(np.float16)

    wq_pad = np.zeros((3, 128, INNER), np.float32)
    wq_pad.reshape(384, INNER)[:DQ] = Wq * SCALE
    wq_pad = np.ascontiguousarray(wq_pad.transpose(1, 0, 2)).astype(np.float16)

    wo_pad = np.zeros((4, 128, DQ), np.float32)
    wo_pad.reshape(512, DQ)[:] = Wo
    wo_pad = np.ascontiguousarray(wo_pad.transpose(1, 0, 2)).astype(np.float16)

    bo_pad = np.zeros((3, 128), np.float32)
    bo_pad.reshape(384)[:DQ] = bo
    bo_pad = np.ascontiguousarray(bo_pad.T)

    in_maps = []
    for c in range(N_CORES):
        rows = slice(c * NS, (c + 1) * NS)
        xt = np.zeros((3, 128, NSB), np.float32)
        xt.reshape(384, NSB)[:DQ, :NS] = x[0, rows].T
        xt.reshape(384, NSB)[:DQ, NS:] = x[1, rows].T
        xt = np.ascontiguousarray(xt.transpose(1, 0, 2)).astype(np.float16)
        # ae[l, h, n] = attn_extra[h, rows[n], l]
        ae = np.ascontiguousarray(
            attn_extra[:, rows, :].transpose(2, 0, 1)).astype(np.float16)
        in_maps.append({
            "xt": xt, "wq": wq_pad, "kt": kt, "v": v16, "wo": wo_pad,
            "bo": bo_pad, "ae": ae,
        })

    import os as _os
    _tc = None
    if _os.environ.get("KERNEL_TRACE_ALL") == "1":
        _tc = list(range(N_CORES))
    res = bass_utils.run_bass_kernel_spmd(
        nc, in_maps, core_ids=list(range(N_CORES)), trace=TRACE, trace_cores=_tc,
    )
    LAST_RESULTS = res

    out = np.empty((2, N, DQ), np.float32)
    for c in range(N_CORES):
        rows = slice(c * NS, (c + 1) * NS)
        yt = res.results[c]["yt"].astype(np.float32)
        out[0, rows] = yt[:, :NS].T
        out[1, rows] = yt[:, NS:].T
    return out
